# revision 26
# baseline (speedup 1.0000x reference)
"""DGCNN (4x EdgeConv + final projection + global max) on 8 Trainium2 cores.

Sharding: data-parallel over batch B=8 -> one point cloud per NeuronCore.

Per-core algorithm (N=2048 points, k=40 neighbors):
  Each EdgeConv layer `h' = max_k lrelu(concat(h_j - h_i, h_i) @ W + b)` is
  decomposed (lrelu monotone, V_i constant over neighbors j):
      U = h @ W_top          (N, d)  fp16, staged to HBM
      V = h @ (W_bot - W_top) + b
      h'[i] = lrelu(max_{j in knn(i)} U[j] + V[i])

  Scores s[i,j] = h_i.h_j - |h_j|^2/2 - |h_i|^2/2 on the PE; the four norm
  rank-1 updates (hi/lo fp16 split, row+col) are fused into ONE rank-4
  matmul per 512-block (cat4l rows [one,one,hi,lo] x cat4r rows
  [hi,lo,one,one]).

  kNN selection per 128-row tile (replaces 15 full-width DVE passes of the
  old max8/max_index/match_replace loop -- each element is now scanned
  twice):
    - 16 chunks of 128: per-chunk top-8 via max8 + max_index.
    - stage 2: the 128 candidates are packed into fp32 keys
      q*2048 + (2047-idx) (q = per-row-scaled quantized value, rounded via
      the +3*2^22 trick), top-40 by 5x max8 + 4x match_replace, original
      indices recovered arithmetically from the winning keys (round to a
      2048-multiple via a mid-binade +1.5*2^34 bias).
  Chunk truncation (a 128-chunk holding >8 of the true top-40) was measured
  end-to-end in numpy at rel_err 0.0070 vs the reference -- below the
  exact-fp16 baseline's 0.0132.

  Gather: one batched SWDGE dma_gather per tile split into 4 calls of 1280
  descriptors on queues 0-3 (queue transfer bandwidth, not descriptor
  count, is the wall).  k-max runs as an in-place log-tree of
  tensor_tensor max ops on the gathered [128, 40, d] block (fp16 2x mode),
  final level writing m_sb directly; the k-reduce of tile t is deferred
  past tile t+1's selection so the DVE never stalls on the gather DMA.
"""

import numpy as np

import concourse.mybir as mybir
import concourse.tile as tile
from concourse import bass_utils, library_config
from concourse.bacc import Bacc
from concourse.masks import make_identity

FP32 = mybir.dt.float32
FP16 = mybir.dt.float16
U16 = mybir.dt.uint16
I16 = mybir.dt.int16

# Problem constants (hardcoded per harness contract)
B = 8
N = 2048
IN_CHAN = 3
H_DIM = [64, 64, 128, 256]
Z_DIM = 512
K = 40
N_CORES = 8

NCH = 12              # score chunks per row (top-8 each -> 96 candidates)
# uneven chunks: 8x171 + 4x170 = 2048 (rel_err 0.0088 measured in numpy)
CH_STARTS = [171 * i for i in range(9)] + [1538, 1708, 1878, 2048]
NCAND = NCH * 8       # 96
BIG_B = float(24 * 2**30)      # 1.5*2^34: mid-binade, multiple of 2048
RND = float(3 * 2**22)         # +RND rounds fp32 in [-2^22,0] to integer

amax = mybir.AluOpType.max
aadd = mybir.AluOpType.add
asub = mybir.AluOpType.subtract
amul = mybir.AluOpType.mult
amin = mybir.AluOpType.min
COPY = mybir.ActivationFunctionType.Copy
IDENT = mybir.ActivationFunctionType.Identity
SQUARE = mybir.ActivationFunctionType.Square


def _dma_gather_raw(nc, out_ap, in_ap, idxs_ap, num_idxs, elem_size,
                    elem_step, queue_num):
    """dma_gather with elem_size_bytes that need not be a 256B multiple.

    The ucode's 256B-alignment assert applies only to transpose mode; in
    plain mode only the source ROW STRIDE (elem_step) must be a 256B
    multiple (descriptor stride field is in 256B units).  bass.dma_gather
    asserts 256B on elem_size unconditionally, so build the instruction
    directly for the 128B-record case (d=64 layers at 256B pitch).
    """
    import concourse.bass_isa  # noqa: F401  (matches bass import env)
    g = nc.gpsimd
    dt_size = mybir.dt.size(in_ap.dtype)
    assert in_ap.dtype == out_ap.dtype
    assert idxs_ap.dtype == I16
    stride_bytes = elem_step * dt_size
    stride_256 = stride_bytes // 256
    assert stride_bytes % 256 == 0 and 0 < stride_256 < 256
    assert in_ap.ap[-1][1] == elem_size
    assert in_ap.ap[0][0] == elem_step
    assert out_ap.ap[0][1] * out_ap.ap[1][1] == num_idxs
    _in_ap = g.lower_ap_dma(in_ap, for_custom_bir_dma=True)
    _idxs_ap = g.lower_ap(idxs_ap)
    _out_ap = g.lower_ap(out_ap)
    return g.add_instruction(
        mybir.InstDMAGatherAnt(
            name=nc.get_next_instruction_name(),
            ins=[*_in_ap, _idxs_ap,
                 g.lower_val_access(g.to_reg(num_idxs))],
            outs=[_out_ap],
            transpose=False,
            num_idxs=num_idxs,
            elem_size=elem_size,
            stride_bytes_256=stride_256,
            gen_mode=0,
            single_packet=True,
            queue_num=queue_num,
            sbuf_tokens_per_rank=0,
            sbuf_free_dim_per_rank=0,
            sbuf_free_dim_pad_per_rank=0,
            sbuf_byte_offset=0,
        )
    )


def build_program(n=N, k=K, in_chan=IN_CHAN, h_dim=None, z2=2 * Z_DIM):
    h_dim = h_dim or H_DIM
    nt = n // 128
    nfb = n // 512
    dmax = max(h_dim)
    cins = [in_chan] + [h for h in h_dim[:-1]]

    nc = Bacc("TRN2", target_bir_lowering=False, debug=False,
              num_devices=N_CORES, num_swdge_queues=4)

    # ---------------- DRAM tensors ----------------
    xT = nc.dram_tensor("xT", [in_chan, n], FP16, kind="ExternalInput")
    uw_d, vw_d, vb_d = [], [], []
    for l in range(4):
        c, d = cins[l], h_dim[l]
        uw_d.append(nc.dram_tensor(f"uw{l}", [c, d], FP16, kind="ExternalInput"))
        vw_d.append(nc.dram_tensor(f"vw{l}", [c, d], FP16, kind="ExternalInput"))
        vb_d.append(nc.dram_tensor(f"vb{l}", [1, d], FP32, kind="ExternalInput"))
    wf_chunk_rows = []
    acc = 0
    for l in range(4):
        d = h_dim[l]
        off = 0
        while off < d:
            rows = min(128, d - off)
            wf_chunk_rows.append((l, off, rows, acc))
            acc += rows
            off += rows
    wf_d = [nc.dram_tensor(f"wf{i}", [rows, z2], FP16, kind="ExternalInput")
            for i, (_, _, rows, _) in enumerate(wf_chunk_rows)]

    # U rows must be a multiple of 256B for dma_gather: pad d=64 to 128 fp16
    u_pad = [max(h_dim[l], 128) for l in range(4)]
    u_dram = [nc.dram_tensor(f"u_scratch{l}", [n, u_pad[l]], FP16,
                             kind="Internal") for l in range(4)]
    iscr_dram = [nc.dram_tensor(f"iscr{j}", [128, k], FP16, kind="Internal")
                 for j in range(2)]
    repmat_d = nc.dram_tensor("repmat", [16, 128], FP16, kind="ExternalInput")
    flipb_d = nc.dram_tensor("flipbase", [128, NCAND], FP32,
                             kind="ExternalInput")
    out_dram = nc.dram_tensor("out", [128, z2 // 128], FP32,
                              kind="ExternalOutput")

    with tile.TileContext(nc) as tc:
        with tc.tile_pool(name="pers", bufs=1) as pers, \
             tc.tile_pool(name="sbuf", bufs=2) as sb, \
             tc.tile_pool(name="gdp", bufs=3) as gdp, \
             tc.tile_pool(name="ps_s", bufs=2, space="PSUM") as ps_s, \
             tc.tile_pool(name="ps_m", bufs=2, space="PSUM") as ps_m:

            # ------------- persistent SBUF -------------
            hT = [pers.tile([max(c, 1), n], FP16, tag=f"hT{l}", name=f"hT{l}")
                  for l, c in enumerate(cins)]
            h4 = [pers.tile([128, n], FP16, tag=f"h4_{j}", name=f"h4_{j}")
                  for j in range(dmax // 128)]
            cat4l = pers.tile([4, n], FP16, tag="c4l", name="c4l")
            cat4r = pers.tile([4, n], FP16, tag="c4r", name="c4r")
            hi16 = pers.tile([1, n], FP16, tag="hi16", name="hi16")
            lo16 = pers.tile([1, n], FP16, tag="lo16", name="lo16")
            sqneg32 = pers.tile([1, n], FP32, tag="sqn", name="sqn")
            onescol32 = pers.tile([128, 1], FP32, tag="oc32", name="oc32")
            ones32 = pers.tile([1, 128], FP32, tag="o32", name="o32")
            rndcol = pers.tile([128, 1], FP32, tag="rnd", name="rnd")
            ident16 = pers.tile([128, 128], FP16, tag="id16", name="id16")
            repmat = pers.tile([16, 128], FP16, tag="repmat", name="repmat")
            flipbase = pers.tile([128, NCAND], FP32, tag="flipb", name="flipb")
            bigB = pers.tile([128, k], FP32, tag="bigB", name="bigB")
            m_sb = pers.tile([128, nt, dmax], FP16, tag="m", name="m")
            # parity-buffered so layer l+1's V staging (emitted inside layer
            # l's per-tile finish) never aliases layer l's V reads
            v_sb2 = [pers.tile([128, nt, dmax], FP16, tag=f"v{j}",
                               name=f"v{j}") for j in range(2)]
            uw = [pers.tile([cins[l], h_dim[l]], FP16, tag=f"uw{l}",
                            name=f"uw{l}") for l in range(4)]
            vw = [pers.tile([cins[l], h_dim[l]], FP16, tag=f"vw{l}",
                            name=f"vw{l}") for l in range(4)]
            vb = [pers.tile([1, h_dim[l]], FP32, tag=f"vb{l}", name=f"vb{l}")
                  for l in range(4)]
            wf = [pers.tile([rows, z2], FP16, tag=f"wf{i}", name=f"wf{i}")
                  for i, (_, _, rows, _) in enumerate(wf_chunk_rows)]
            red = pers.tile([128, (z2 // 128) * nfb], FP32, tag="red",
                            name="red")
            out_sb = pers.tile([128, z2 // 128], FP32, tag="out_sb",
                               name="out_sb")

            # ------------- stage inputs -------------
            nc.gpsimd.load_library(library_config.mlp)
            nc.sync.dma_start(repmat[:], repmat_d.ap())
            nc.sync.dma_start(flipbase[:], flipb_d.ap())
            nc.sync.dma_start(hT[0][:in_chan, :], xT.ap())
            for l in range(4):
                nc.sync.dma_start(uw[l][:], uw_d[l].ap())
                nc.sync.dma_start(vw[l][:], vw_d[l].ap())
                nc.sync.dma_start(vb[l][:], vb_d[l].ap())
            for i in range(len(wf)):
                nc.sync.dma_start(wf[i][:], wf_d[i].ap())
            nc.vector.memset(onescol32[:], 1.0)
            nc.vector.memset(ones32[:], 1.0)
            # rows 2-3 of cat4l / 0-1 of cat4r are overwritten per layer
            nc.vector.memset(cat4l[:], 1.0)        # lhs rows: one,one,hi,lo
            nc.vector.memset(cat4r[:], 1.0)        # rhs rows: hi,lo,one,one
            nc.vector.memset(bigB[:], BIG_B)
            nc.vector.memset(rndcol[:], RND)
            make_identity(nc, ident16[:])

            # final projection for one 512-col block of points (invoked from
            # layer 3's per-tile finish once its 4 h4 tiles are ready)
            h_bufs = {0: hT[1][:h_dim[0], :], 1: hT[2][:h_dim[1], :],
                      2: hT[3][:h_dim[2], :]}
            nmb = z2 // 128

            def emit_proj(fb):
                fs = slice(fb * 512, (fb + 1) * 512)
                for mb in range(nmb):
                    ms = slice(mb * 128, (mb + 1) * 128)
                    p_f = ps_s.tile([128, 512], FP32, tag="s", name="s")
                    for i, (wl, off, rows, _) in enumerate(wf_chunk_rows):
                        if wl < 3:
                            rhs = h_bufs[wl][off:off + rows, fs]
                        else:
                            rhs = h4[off // 128][:rows, fs]
                        nc.tensor.matmul(p_f[:], lhsT=wf[i][:, ms], rhs=rhs,
                                         start=(i == 0),
                                         stop=(i == len(wf_chunk_rows) - 1))
                    nc.vector.tensor_reduce(
                        out=red[:, mb * nfb + fb:mb * nfb + fb + 1],
                        in_=p_f[:], axis=mybir.AxisListType.X, op=amax)

            # ------------- EdgeConv layers -------------
            for l in range(4):
                c, d = cins[l], h_dim[l]
                ht = hT[l][:c, :]

                # column norms: sqneg = -|h_j|^2/2 (fp32), hi/lo fp16 split
                l2sq = sb.tile([128, n], FP32, tag="l2sq", name="l2sq")
                nc.scalar.activation(l2sq[:c, :], ht, SQUARE)
                for fb in range(nfb):
                    fs = slice(fb * 512, (fb + 1) * 512)
                    p_q = ps_m.tile([128, 512], FP32, tag="misc", name="misc")
                    nc.tensor.matmul(p_q[:1, :], lhsT=onescol32[:c, :],
                                     rhs=l2sq[:c, fs], start=True, stop=True)
                    nc.scalar.activation(sqneg32[:, fs], p_q[:1, :], COPY,
                                         scale=-0.5)
                nc.scalar.copy(hi16[:], sqneg32[:])
                nc.vector.tensor_tensor(out=lo16[:], in0=sqneg32[:],
                                        in1=hi16[:], op=asub)
                # engine writes must start at partition 0; sb2sb DMA is free
                # of that restriction, so place hi/lo rows 2-3 / 0-1 via DMA
                nc.sync.dma_start(cat4l[2:3, :], hi16[:])
                nc.sync.dma_start(cat4l[3:4, :], lo16[:])
                nc.sync.dma_start(cat4r[0:1, :], hi16[:])
                nc.sync.dma_start(cat4r[1:2, :], lo16[:])

                # U (staged to HBM for the gather) / V for one tile.  Layer
                # 0 emits these up front; layers 1-3 have them emitted inside
                # the PREVIOUS layer's per-tile finish, right after that
                # tile's hT columns are transposed, so u_dram is ready well
                # before the layer starts.
                def emit_uv(ll, tb):
                    dd = h_dim[ll]
                    hts = hT[ll][:cins[ll], tb * 128:(tb + 1) * 128]
                    p_u = ps_m.tile([128, 512], FP32, tag="misc", name="misc")
                    nc.tensor.matmul(p_u[:, :dd], lhsT=hts, rhs=uw[ll][:],
                                     start=True, stop=True)
                    ustage = sb.tile([128, dmax], FP16, tag="ustage",
                                     name="ustage")
                    nc.scalar.copy(ustage[:, :dd], p_u[:, :dd])
                    nc.sync.dma_start(
                        u_dram[ll].ap().rearrange("(t p) d -> t p d",
                                                  p=128)[tb][:, :dd],
                        ustage[:, :dd])
                    p_v = ps_m.tile([128, 512], FP32, tag="misc", name="misc")
                    nc.tensor.matmul(p_v[:, :dd], lhsT=hts, rhs=vw[ll][:],
                                     start=True, stop=False)
                    nc.tensor.matmul(p_v[:, :dd], lhsT=ones32[:],
                                     rhs=vb[ll][:], start=False, stop=True)
                    nc.scalar.copy(v_sb2[ll % 2][:, tb, :dd], p_v[:, :dd])

                if l == 0:
                    for tb in range(nt):
                        emit_uv(0, tb)
                v_sb = v_sb2[l % 2]

                # scores + chunked top-k + gather per tile; the k-reduce of
                # tile t is deferred past tile t+1's selection so the DVE
                # never stalls on the gather DMA.
                pending = None
                for tb in range(nt):
                    bs = slice(tb * 128, (tb + 1) * 128)
                    s_sb = sb.tile([128, n], FP16, tag="s_sb", name="s_sb")
                    for fb in range(nfb):
                        fs = slice(fb * 512, (fb + 1) * 512)
                        p_s = ps_s.tile([128, 512], FP32, tag="s", name="s")
                        nc.tensor.matmul(p_s[:], lhsT=ht[:, bs], rhs=ht[:, fs],
                                         start=True, stop=False)
                        nc.tensor.matmul(p_s[:], lhsT=cat4l[:, bs],
                                         rhs=cat4r[:, fs], start=False,
                                         stop=True)
                        nc.scalar.activation(s_sb[:, fs], p_s[:], COPY)

                    # per-chunk top-8 values + local indices
                    cvals = sb.tile([128, NCH, 8], FP16, tag="cv", name="cv")
                    cidx = sb.tile([128, NCH, 8], U16, tag="ci", name="ci")
                    for ch in range(NCH):
                        cs = slice(CH_STARTS[ch], CH_STARTS[ch + 1])
                        nc.vector.max(out=cvals[:, ch, :], in_=s_sb[:, cs])
                    for ch in range(NCH):
                        cs = slice(CH_STARTS[ch], CH_STARTS[ch + 1])
                        nc.vector.max_index(out=cidx[:, ch, :],
                                            in_max=cvals[:, ch, :],
                                            in_values=s_sb[:, cs])

                    # stage 2: quantize + pack (q*2048 + 2047 - orig_idx)
                    cvflat = cvals[:].rearrange("p a b -> p (a b)")
                    minc = sb.tile([128, 1], FP32, tag="minc", name="minc")
                    nc.vector.tensor_reduce(out=minc[:], in_=cvflat,
                                            axis=mybir.AxisListType.X, op=amin)
                    sig = sb.tile([128, 1], FP32, tag="sig", name="sig")
                    nc.vector.tensor_scalar(out=sig[:], in0=minc[:],
                                            scalar1=-1.0 / 2046.0,
                                            scalar2=None, op0=amul)
                    nc.vector.reciprocal(out=sig[:], in_=sig[:])
                    # qs = cvals*sig + 3*2^22: the fp32 write rounds to int
                    qs = sb.tile([128, NCAND], FP32, tag="qs", name="qs")
                    nc.scalar.activation(qs[:], cvflat, IDENT, scale=sig[:],
                                         bias=rndcol[:])
                    q2048 = sb.tile([128, NCAND], FP32, tag="q2", name="q2")
                    nc.vector.tensor_scalar(out=q2048[:], in0=qs[:],
                                            scalar1=-RND, scalar2=2048.0,
                                            op0=aadd, op1=amul)
                    qb = sb.tile([128, NCAND], FP32, tag="qb", name="qb")
                    nc.vector.tensor_tensor(out=qb[:], in0=q2048[:],
                                            in1=flipbase[:], op=aadd)
                    vpack = sb.tile([128, NCAND], FP32, tag="vp", name="vp")
                    nc.vector.scalar_tensor_tensor(
                        out=vpack[:],
                        in0=cidx[:].rearrange("p a b -> p (a b)"), scalar=-1.0,
                        in1=qb[:], op0=amul, op1=aadd)

                    # top-40 of packed keys
                    w40 = sb.tile([128, 5, 8], FP32, tag="w40", name="w40")
                    for r in range(5):
                        nc.vector.max(out=w40[:, r, :], in_=vpack[:])
                        if r < 4:
                            nc.vector.match_replace(
                                out=vpack[:], in_to_replace=w40[:, r, :],
                                in_values=vpack[:], imm_value=-3.0e38)
                    w40f = w40[:].rearrange("p a b -> p (a b)")

                    # recover original indices from keys
                    e1 = sb.tile([128, k], FP32, tag="e1", name="e1")
                    nc.vector.scalar_tensor_tensor(
                        out=e1[:], in0=w40f, scalar=-1023.5, in1=bigB[:],
                        op0=aadd, op1=aadd)
                    qw = sb.tile([128, k], FP32, tag="qw", name="qw")
                    nc.vector.tensor_scalar(out=qw[:], in0=e1[:],
                                            scalar1=-BIG_B, scalar2=None,
                                            op0=aadd)
                    orig = sb.tile([128, k], FP32, tag="orig", name="orig")
                    nc.vector.scalar_tensor_tensor(
                        out=orig[:], in0=qw[:], scalar=2047.0, in1=w40f,
                        op0=aadd, op1=asub)
                    idx16 = sb.tile([128, k], FP16, tag="idx16", name="idx16")
                    nc.vector.tensor_copy(out=idx16[:], in_=orig[:])

                    # relayout into dma_gather's 16-partition wrap, replicated
                    jb = tb % 2
                    nc.sync.dma_start(iscr_dram[jb].ap(), idx16[:])
                    jm = sb.tile([16, 8, k], FP16, tag="jm", name="jm")
                    nc.sync.dma_start(
                        jm[:], iscr_dram[jb].ap().rearrange(
                            "(j q) t -> q j t", q=16))
                    p_w = ps_m.tile([128, 8 * k], FP32, tag="wrap", name="wrap")
                    nc.tensor.matmul(p_w[:], lhsT=repmat[:],
                                     rhs=jm[:].rearrange("q j t -> q t j"),
                                     start=True, stop=True)
                    iwrap = sb.tile([128, 8 * k], I16, tag="iwrap",
                                    name="iwrap")
                    nc.vector.tensor_copy(out=iwrap[:], in_=p_w[:])

                    # gather: 5 calls of 1024 descriptors (SWDGE ring limit),
                    # queue rotated per tile so all 4 queues carry equal
                    # bytes.  d=64 layers gather 128B records at 256B pitch.
                    gflat = gdp.tile([128, k * 256], FP16, tag="gd",
                                     name="gdest")
                    gdest = gflat[:, :k * d].rearrange(
                        "p (a r) -> p a r", r=d)
                    for ch5 in range(5):
                        _dma_gather_raw(
                            nc, gdest[:, 8 * ch5:8 * ch5 + 8, :],
                            u_dram[l].ap()[:, :d],
                            iwrap[:, 64 * ch5:64 * ch5 + 64],
                            num_idxs=1024, elem_size=d, elem_step=u_pad[l],
                            queue_num=(ch5 + tb) % 4)
                    def _finish_tile(pt, pg):
                        # k-max, +V, lrelu, transpose for tile pt (emitted
                        # after tile pt+1's gather so the DVE never stalls)
                        _kmax_tree(nc, pg, m_sb[:, pt, :d], d)
                        nc.vector.tensor_tensor(
                            out=m_sb[:, pt, :d], in0=m_sb[:, pt, :d],
                            in1=v_sb[:, pt, :d], op=aadd)
                        nc.vector.scalar_tensor_tensor(
                            out=m_sb[:, pt, :d], in0=m_sb[:, pt, :d],
                            scalar=0.2, in1=m_sb[:, pt, :d], op0=amul,
                            op1=amax)
                        pbs = slice(pt * 128, (pt + 1) * 128)
                        for dc in range((d + 127) // 128):
                            rows = min(128, d - dc * 128)
                            p_t = ps_m.tile([128, 128], FP16, tag="tr",
                                            name="tr")
                            nc.tensor.transpose(
                                p_t[:rows, :128],
                                in_=m_sb[:, pt, dc * 128:dc * 128 + rows],
                                identity=ident16[:])
                            if l < 3:
                                dstt = hT[l + 1][dc * 128:dc * 128 + rows, pbs]
                            else:
                                dstt = h4[dc][:rows, pbs]
                            nc.scalar.copy(dstt, p_t[:rows, :128])
                        if l < 3:
                            emit_uv(l + 1, pt)
                        elif pt % 4 == 3:
                            # final projection for the 512-col block whose 4
                            # h4 tiles just completed -- overlaps the rest of
                            # layer 3 instead of serializing at the end
                            emit_proj(pt // 4)

                    if pending is not None:
                        _finish_tile(*pending)
                    pending = (tb, gdest)
                _finish_tile(*pending)

            # ------------- final global max -------------
            nc.vector.tensor_reduce(
                out=out_sb[:],
                in_=red[:].rearrange("p (m f) -> p m f", f=nfb),
                axis=mybir.AxisListType.X, op=amax)
            nc.sync.dma_start(out_dram.ap(), out_sb[:])

    nc.compile()
    return nc


def _kmax_tree(nc, g, dst, d):
    """In-place log-tree max over k=40 on gathered [128, 40, d] fp16;
    writes the [128, d] result into dst."""
    def gv(a, b):
        return g[:, a:b, :d]
    nc.vector.tensor_tensor(out=gv(0, 20), in0=gv(0, 20), in1=gv(20, 40),
                            op=amax)
    nc.vector.tensor_tensor(out=gv(0, 10), in0=gv(0, 10), in1=gv(10, 20),
                            op=amax)
    nc.vector.tensor_tensor(out=gv(0, 5), in0=gv(0, 5), in1=gv(5, 10),
                            op=amax)
    if d <= 128:
        # single strided reduce over the 5 survivors
        nc.vector.tensor_reduce(
            out=dst, in_=gv(0, 5).rearrange("p a d -> p d a"),
            axis=mybir.AxisListType.X, op=amax)
    else:
        nc.vector.tensor_tensor(out=gv(0, 2), in0=gv(0, 2), in1=gv(2, 4),
                                op=amax)
        nc.vector.tensor_tensor(out=g[:, 0, :d], in0=g[:, 0, :d],
                                in1=g[:, 1, :d], op=amax)
        nc.vector.tensor_tensor(out=dst, in0=g[:, 0, :d], in1=g[:, 4, :d],
                                op=amax)


def _prep_core_inputs(x_c, params, h_dim, in_chan):
    """Host-side input prep for one core: transpose x, split/derive weights."""
    cins = [in_chan] + [h for h in h_dim[:-1]]
    m = {"xT": np.ascontiguousarray(x_c.T).astype(np.float16)}
    m["repmat"] = (np.arange(128)[None, :] % 16 ==
                   np.arange(16)[:, None]).astype(np.float16)
    # flipbase[p, c] = 2047 - chunk_base(c); candidate c is rank c%8 of
    # chunk c//8
    starts = np.array(CH_STARTS[:-1], np.float32)
    fb = 2047.0 - starts[np.arange(NCAND) // 8]
    m["flipbase"] = np.broadcast_to(fb[None, :], (128, NCAND)).astype(
        np.float32)
    for l in range(4):
        c = cins[l]
        w = params[f"W{l}"]
        m[f"uw{l}"] = np.ascontiguousarray(w[:c]).astype(np.float16)
        m[f"vw{l}"] = np.ascontiguousarray(w[c:] - w[:c]).astype(np.float16)
        m[f"vb{l}"] = params[f"b{l}"][None, :].astype(np.float32)
    wfull = params["Wf"]
    i = 0
    acc = 0
    for l in range(4):
        d = h_dim[l]
        off = 0
        while off < d:
            rows = min(128, d - off)
            m[f"wf{i}"] = np.ascontiguousarray(
                wfull[acc:acc + rows]).astype(np.float16)
            acc += rows
            off += rows
            i += 1
    return m


_NC_CACHE = {}


def kernel(**inputs):
    x = np.asarray(inputs["x"], dtype=np.float32)
    params = {k_: np.asarray(v, dtype=np.float32) for k_, v in inputs.items()
              if k_ != "x"}

    if "nc" not in _NC_CACHE:
        _NC_CACHE["nc"] = build_program()
    nc = _NC_CACHE["nc"]

    in_maps = [_prep_core_inputs(x[c], params, H_DIM, IN_CHAN)
               for c in range(B)]
    res = bass_utils.run_bass_kernel_spmd(nc, in_maps,
                                          core_ids=list(range(N_CORES)))
    bf = params["bf"]
    out = np.stack([res.results[c]["out"].T.ravel() for c in range(B)])
    return (out + bf[None, :]).astype(np.float32)
